# revision 9
# baseline (speedup 1.0000x reference)
"""VGCN encoder (2-layer GCN, shared normalized adjacency) on 8 Trainium2 cores.

Strategy: node-partitioned graph parallelism. Nodes are padded to
NPAD = 8*49*128 and core c owns the 49 node-tiles [49c, 49c+49). All edges
(plus one self-edge per node, which realizes both GCN's +1 degree and the
self-loop term) are routed to the core owning their dst node, bucketed by
dst tile, and aggregated on-device with one-hot matmuls:

    agg[dst_tile] += S.T @ us[src]   (S[e, j] = dst_local[e] == j, built on DVE)

Messages are fetched with SWDGE dma_gather from a DRAM table whose rows are
stored in a (core, partition, tile)-major permutation so every bulk table
write is a full-rate contiguous DMA; the host permutes gather indices to
match. dis = 1/sqrt(deg) and the layer-1 activations are exchanged with
AllGather collectives. Weight matrices are replicated.
"""

import sys

sys.path.insert(0, "/opt/trn_rl_repo")

import numpy as np

from concourse import bacc, mybir, tile
from concourse.bass_utils import run_bass_kernel_spmd
from concourse.masks import make_identity

F32 = mybir.dt.float32
BF16 = mybir.dt.bfloat16
I16 = mybir.dt.int16
I32 = mybir.dt.int32


class Cfg:
    def __init__(self, n=50000, e=800000, in_dim=128, hid=64, ncores=8,
                 nbuck=49, half=32768, chunk_tiles=32, sbatch=8):
        self.N, self.E, self.IN, self.HID = n, e, in_dim, hid
        self.NCORES, self.NBUCK = ncores, nbuck
        self.P = 128
        self.SHARD = nbuck * 128
        self.NPAD = ncores * self.SHARD
        self.NTILES = ncores * nbuck
        self.HALF = half          # gather-table split so int16 indices fit
        self.CH = chunk_tiles     # tiles (of 128 rows) per dma_gather call
        self.SB = sbatch          # tiles per batched one-hot build
        assert self.NPAD >= n and half <= 32768


DEFAULT = Cfg()


def _row_of_node(n, cfg):
    """Table row of node n in the (core, partition, tile)-major layout."""
    c, r = n // cfg.SHARD, n % cfg.SHARD
    t, p = r // 128, r % 128
    return c * cfg.SHARD + p * cfg.NBUCK + t


def build_layout(edge_index, cfg=DEFAULT):
    """Static per-core edge streams. Returns dict with per-core host arrays
    plus the (identical-across-cores) tile structure."""
    src = np.asarray(edge_index[0], np.int64)
    dst = np.asarray(edge_index[1], np.int64)

    per_core = []
    cnts = np.zeros((cfg.NCORES, cfg.NBUCK * 2), np.int64)
    for c in range(cfg.NCORES):
        m = (dst >= c * cfg.SHARD) & (dst < (c + 1) * cfg.SHARD)
        s = src[m]
        d = dst[m]
        selfn = np.arange(c * cfg.SHARD, (c + 1) * cfg.SHARD, dtype=np.int64)
        s = np.concatenate([s, selfn])
        d = np.concatenate([d, selfn])
        row = _row_of_node(s, cfg) if False else None
        # permuted table row for each source node
        cc, rr = s // cfg.SHARD, s % cfg.SHARD
        row = cc * cfg.SHARD + (rr % 128) * cfg.NBUCK + rr // 128
        dr = d - c * cfg.SHARD
        b = dr >> 7
        dl = dr & 127
        h = (row >= cfg.HALF).astype(np.int64)
        key = b * 2 + h
        order = np.argsort(key, kind="stable")
        row, dl, key = row[order], dl[order], key[order]
        per_core.append((row, dl, key))
        cnts[c] = np.bincount(key, minlength=cfg.NBUCK * 2)

    ntile = np.ceil(cnts.max(axis=0) / 128.0).astype(np.int64)  # [98]
    ntA, ntB = ntile[0::2], ntile[1::2]
    nTA, nTB = int(ntA.sum()), int(ntB.sum())

    # tile -> bucket map per half (same on every core)
    tbA = np.repeat(np.arange(cfg.NBUCK), ntA)
    tbB = np.repeat(np.arange(cfg.NBUCK), ntB)

    offA = np.concatenate([[0], np.cumsum(ntA)]) * 128
    offB = np.concatenate([[0], np.cumsum(ntB)]) * 128

    cores = []
    for c in range(cfg.NCORES):
        row, dl, key = per_core[c]
        bounds = np.searchsorted(key, np.arange(cfg.NBUCK * 2 + 1))
        idxA = np.zeros(nTA * 128, np.int64)
        dlA = np.full(nTA * 128, 128, np.int64)
        idxB = np.zeros(nTB * 128, np.int64)
        dlB = np.full(nTB * 128, 128, np.int64)
        for b in range(cfg.NBUCK):
            lo, hi = bounds[2 * b], bounds[2 * b + 1]
            o = offA[b]
            idxA[o:o + hi - lo] = row[lo:hi]
            dlA[o:o + hi - lo] = dl[lo:hi]
            lo, hi = bounds[2 * b + 1], bounds[2 * b + 2]
            o = offB[b]
            idxB[o:o + hi - lo] = row[lo:hi] - cfg.HALF
            dlB[o:o + hi - lo] = dl[lo:hi]

        def wrap(stream):
            a = stream.reshape(-1, 16).T.astype(np.int16)  # [16, L/16]
            return np.tile(a, (8, 1))                      # replicate across q7 cores

        cores.append(dict(
            idxA=wrap(idxA), idxB=wrap(idxB),
            dlA=np.ascontiguousarray(dlA.reshape(-1, 128).T.astype(np.float32)),
            dlB=np.ascontiguousarray(dlB.reshape(-1, 128).T.astype(np.float32)),
        ))

    return dict(ntA=tuple(int(x) for x in ntA), ntB=tuple(int(x) for x in ntB),
                tbA=tbA, tbB=tbB, nTA=nTA, nTB=nTB, cores=cores)


def build_program(layout, cfg=DEFAULT, has_bias=False):
    """Emit the SPMD bass program (identical on all cores)."""
    nc = bacc.Bacc("TRN2", target_bir_lowering=False, debug=False,
                   num_devices=cfg.NCORES)
    P, NB, HID = cfg.P, cfg.NBUCK, cfg.HID
    nTA, nTB = layout["nTA"], layout["nTB"]
    tb = {0: layout["tbA"], 1: layout["tbB"]}
    nT = {0: nTA, 1: nTB}
    HALVES = [H for H in (0, 1) if nT[H] > 0]

    # ---------------- I/O ----------------
    xT_in = nc.dram_tensor("xT", [P, cfg.NPAD], F32, kind="ExternalInput")
    w1_in = nc.dram_tensor("w1", [cfg.IN, HID], F32, kind="ExternalInput")
    wmu_in = nc.dram_tensor("wmu", [HID, HID], F32, kind="ExternalInput")
    wlv_in = nc.dram_tensor("wlv", [HID, HID], F32, kind="ExternalInput")
    idx_name = {0: "idxA", 1: "idxB"}
    dl_name = {0: "dlA", 1: "dlB"}
    idx_in = {H: nc.dram_tensor(idx_name[H], [P, nT[H] * 8], I16, kind="ExternalInput")
              for H in HALVES}
    dl_in = {H: nc.dram_tensor(dl_name[H], [P, nT[H]], F32, kind="ExternalInput")
             for H in HALVES}
    if has_bias:
        b1_in = nc.dram_tensor("b1", [1, HID], F32, kind="ExternalInput")
        bmu_in = nc.dram_tensor("bmu", [1, HID], F32, kind="ExternalInput")
        blv_in = nc.dram_tensor("blv", [1, HID], F32, kind="ExternalInput")
    zmu_out = nc.dram_tensor("zmu", [P, NB, HID], F32, kind="ExternalOutput")
    zlv_out = nc.dram_tensor("zlv", [P, NB, HID], F32, kind="ExternalOutput")

    with tile.TileContext(nc) as tc:
        import contextlib
        stack = contextlib.ExitStack()
        with stack:
            dram = stack.enter_context(tc.tile_pool(name="dram", bufs=1, space="DRAM"))
            cpool = stack.enter_context(tc.tile_pool(name="const", bufs=1))

            us_tab = dram.tile([cfg.NPAD, HID], F32)
            hs2_bnc = dram.tile([cfg.SHARD, HID], F32)
            hs2_tab = dram.tile([cfg.NPAD, HID], F32)
            dis_bnc = dram.tile([P, NB], F32)
            dis_ag = dram.tile([cfg.NCORES * P, NB], F32)

            # ------------- constants / small loads -------------
            w1_sb = cpool.tile([cfg.IN, HID], F32)
            nc.sync.dma_start(out=w1_sb[:], in_=w1_in.ap()[:])
            wmu_sb = cpool.tile([HID, HID], F32)
            nc.sync.dma_start(out=wmu_sb[:], in_=wmu_in.ap()[:])
            wlv_sb = cpool.tile([HID, HID], F32)
            nc.sync.dma_start(out=wlv_sb[:], in_=wlv_in.ap()[:])

            iota_i = cpool.tile([P, P], I32)
            nc.gpsimd.iota(iota_i[:], pattern=[[1, P]], base=0, channel_multiplier=0)
            iota_f = cpool.tile([P, P], F32)
            nc.vector.tensor_copy(out=iota_f[:], in_=iota_i[:])
            iota_b = cpool.tile([P, P], BF16)
            nc.vector.tensor_copy(out=iota_b[:], in_=iota_i[:])

            ident = cpool.tile([P, P], F32)
            make_identity(nc, ident[:])

            # deg-matmul stationary: sliding one-hot window, col 48 == 1
            slide = cpool.tile([P, 2 * NB - 1], BF16)
            nc.vector.memset(slide[:], 0)
            nc.vector.memset(slide[:, NB - 1:NB], 1.0)

            idx_sb, dl_sb, dl_bf = {}, {}, {}
            for H in HALVES:
                idx_sb[H] = cpool.tile([P, nT[H] * 8], I16, tag=f"idx{H}", name=f"idx{H}")
                nc.sync.dma_start(out=idx_sb[H][:], in_=idx_in[H].ap()[:])
                dl_sb[H] = cpool.tile([P, nT[H]], F32, tag=f"dl{H}", name=f"dls{H}")
                nc.sync.dma_start(out=dl_sb[H][:], in_=dl_in[H].ap()[:])
                dl_bf[H] = cpool.tile([P, nT[H]], BF16, tag=f"dlb{H}", name=f"dlb{H}")
                nc.vector.tensor_copy(out=dl_bf[H][:], in_=dl_sb[H][:])

            if has_bias:
                brow = cpool.tile([1, 3 * HID], F32)
                nc.sync.dma_start(out=brow[:, 0:HID], in_=b1_in.ap()[:])
                nc.sync.dma_start(out=brow[:, HID:2 * HID], in_=bmu_in.ap()[:])
                nc.sync.dma_start(out=brow[:, 2 * HID:], in_=blv_in.ap()[:])
                bias_bc = cpool.tile([P, 3 * HID], F32)
                nc.gpsimd.partition_broadcast(bias_bc[:], brow[:])

            def build_S(spool, H, dtype, tag):
                """Batched one-hot builds for a whole half-stream."""
                tiles = []
                dlt = dl_bf[H] if dtype == BF16 else dl_sb[H]
                iot = iota_b if dtype == BF16 else iota_f
                for t0 in range(0, nT[H], cfg.SB):
                    tn = min(cfg.SB, nT[H] - t0)
                    st = spool.tile([P, cfg.SB, P], dtype, tag=tag, name=f"S{tag}")
                    nc.vector.tensor_tensor(
                        out=st[:, :tn, :],
                        in0=dlt[:, t0:t0 + tn].to_broadcast([P, tn, P]),
                        in1=iot[:, None, :].to_broadcast([P, tn, P]),
                        op=mybir.AluOpType.is_equal,
                    )
                    tiles.append(st)
                return lambda t: tiles[t // cfg.SB][:, t % cfg.SB, :]

            def gather_chunks(mpool, H, table, tag):
                tiles = []
                for t0 in range(0, nT[H], cfg.CH):
                    tn = min(cfg.CH, nT[H] - t0)
                    mt = mpool.tile([P, cfg.CH, HID], F32, tag=tag, name=f"M{tag}")
                    nc.gpsimd.dma_gather(
                        out_ap=mt[:, :tn, :],
                        in_ap=(table[:min(cfg.HALF, cfg.NPAD), :] if H == 0
                           else table[cfg.HALF:, :]),
                        idxs_ap=idx_sb[H][:, t0 * 8:(t0 + tn) * 8],
                        num_idxs=tn * 128, num_idxs_reg=tn * 128,
                        elem_size=HID, single_packet=(tn * 128 <= 512),
                    )
                    tiles.append(mt)
                return lambda t: tiles[t // cfg.CH][:, t % cfg.CH, :]

            # bucket -> [(half, tile)] consumption order
            entries = [[] for _ in range(NB)]
            for H in HALVES:
                for t, b in enumerate(tb[H]):
                    entries[int(b)].append((H, t))

            # ================= PHASE A: deg + u -> us table =================
            with tc.tile_pool(name="sdeg", bufs=3) as sdeg, \
                 tc.tile_pool(name="pdeg", bufs=1, space="PSUM") as pdeg, \
                 tc.tile_pool(name="xa", bufs=3) as xa, \
                 tc.tile_pool(name="pu", bufs=4, space="PSUM") as pu, \
                 tc.tile_pool(name="usb", bufs=1) as usb, \
                 tc.tile_pool(name="misc", bufs=2) as misc:

                S_deg = {H: build_S(sdeg, H, BF16, f"sb{H}") for H in HALVES}
                degps = pdeg.tile([NB, P], F32, space="PSUM")
                seq = [(H, t) for H in HALVES for t in range(nT[H])]
                for i, (H, t) in enumerate(seq):
                    b = int(tb[H][t])
                    nc.tensor.matmul(
                        out=degps[:],
                        lhsT=slide[:, NB - 1 - b:2 * NB - 1 - b],
                        rhs=S_deg[H](t),
                        start=(i == 0), stop=(i == len(seq) - 1),
                    )
                deg_sb = misc.tile([NB, P], F32)
                nc.vector.tensor_copy(out=deg_sb[:], in_=degps[:])
                degT_ps = pu.tile([P, NB], F32, space="PSUM", tag="dtp", bufs=1)
                nc.tensor.transpose(out=degT_ps[:], in_=deg_sb[:],
                                    identity=ident[:NB, :NB])
                sq = misc.tile([P, NB], F32)
                nc.scalar.sqrt(out=sq[:], in_=degT_ps[:])
                dis_own = cpool.tile([P, NB], F32)
                nc.vector.reciprocal(out=dis_own[:], in_=sq[:])

                dis_full = cpool.tile([P, cfg.NTILES], F32)
                if cfg.NCORES > 1:
                    nc.sync.dma_start(out=dis_bnc[:], in_=dis_own[:])
                    nc.gpsimd.collective_compute(
                        "AllGather", mybir.AluOpType.bypass,
                        replica_groups=[list(range(cfg.NCORES))],
                        ins=[dis_bnc.opt()], outs=[dis_ag.opt()],
                    )
                    for c2 in range(cfg.NCORES):
                        nc.sync.dma_start(out=dis_full[:, c2 * NB:(c2 + 1) * NB],
                                          in_=dis_ag[c2 * P:(c2 + 1) * P, :])
                else:
                    nc.vector.tensor_copy(out=dis_full[:], in_=dis_own[:])

                us_sb = usb.tile([P, cfg.NTILES, HID], F32)
                for T in range(cfg.NTILES):
                    xt = xa.tile([P, P], F32, tag="xt")
                    nc.sync.dma_start(out=xt[:], in_=xT_in.ap()[:, T * P:(T + 1) * P])
                    ups = pu.tile([P, HID], F32, space="PSUM", tag="u")
                    nc.tensor.matmul(out=ups[:], lhsT=xt[:], rhs=w1_sb[:],
                                     start=True, stop=True)
                    nc.vector.tensor_scalar(
                        out=us_sb[:, T, :], in0=ups[:],
                        scalar1=dis_full[:, T:T + 1], scalar2=None,
                        op0=mybir.AluOpType.mult)
                usv = us_sb[:].rearrange("p (c t) f -> p c t f", c=cfg.NCORES)
                tabv = us_tab[:].rearrange("(c p t) f -> p c t f",
                                           c=cfg.NCORES, p=P)
                for c2 in range(cfg.NCORES):
                    nc.sync.dma_start(out=tabv[:, c2], in_=usv[:, c2])

            # ================= PHASE B: layer-1 aggregation =================
            with tc.tile_pool(name="sl1", bufs=3) as sl1, \
                 tc.tile_pool(name="msg1", bufs=2) as msg1, \
                 tc.tile_pool(name="pagg", bufs=4, space="PSUM") as pagg, \
                 tc.tile_pool(name="hb", bufs=2) as hb, \
                 tc.tile_pool(name="hs2b", bufs=1) as hs2b:

                msg = {H: gather_chunks(msg1, H, us_tab[:], f"m{H}") for H in HALVES}
                S1 = {H: build_S(sl1, H, F32, f"s{H}") for H in HALVES}
                hs2_sb = hs2b.tile([P, NB, HID], F32)
                for b in range(NB):
                    ps = pagg.tile([P, HID], F32, space="PSUM", tag="agg")
                    ent = entries[b]
                    for i, (H, t) in enumerate(ent):
                        nc.tensor.matmul(out=ps[:], lhsT=S1[H](t), rhs=msg[H](t),
                                         start=(i == 0), stop=(i == len(ent) - 1))
                    h_sb = hb.tile([P, HID], F32, tag="h")
                    if has_bias:
                        nc.vector.tensor_scalar(
                            out=h_sb[:], in0=ps[:],
                            scalar1=dis_own[:, b:b + 1], scalar2=None,
                            op0=mybir.AluOpType.mult)
                        nc.vector.tensor_tensor(out=h_sb[:], in0=h_sb[:],
                                                in1=bias_bc[:, 0:HID],
                                                op=mybir.AluOpType.add)
                        nc.vector.tensor_relu(out=h_sb[:], in_=h_sb[:])
                    else:
                        nc.scalar.activation(
                            out=h_sb[:], in_=ps[:],
                            func=mybir.ActivationFunctionType.Relu,
                            scale=dis_own[:, b:b + 1])
                    nc.vector.tensor_scalar(
                        out=hs2_sb[:, b, :], in0=h_sb[:],
                        scalar1=dis_own[:, b:b + 1], scalar2=None,
                        op0=mybir.AluOpType.mult)
                if cfg.NCORES > 1:
                    nc.sync.dma_start(
                        out=hs2_bnc[:].rearrange("(p t) f -> p t f", p=P),
                        in_=hs2_sb[:])
                    nc.gpsimd.collective_compute(
                        "AllGather", mybir.AluOpType.bypass,
                        replica_groups=[list(range(cfg.NCORES))],
                        ins=[hs2_bnc.opt()], outs=[hs2_tab.opt()],
                    )
                else:
                    nc.sync.dma_start(
                        out=hs2_tab[:].rearrange("(p t) f -> p t f", p=P),
                        in_=hs2_sb[:])

            # ================= PHASE C: layer-2 + projections =================
            with tc.tile_pool(name="sl2", bufs=3) as sl2, \
                 tc.tile_pool(name="msg2", bufs=2) as msg2, \
                 tc.tile_pool(name="pagg2", bufs=2, space="PSUM") as pagg2, \
                 tc.tile_pool(name="ptr", bufs=2, space="PSUM") as ptr, \
                 tc.tile_pool(name="pproj", bufs=1, space="PSUM") as pproj, \
                 tc.tile_pool(name="pz", bufs=1, space="PSUM") as pz, \
                 tc.tile_pool(name="l2sb", bufs=3) as l2sb, \
                 tc.tile_pool(name="zb", bufs=1) as zb:

                msg = {H: gather_chunks(msg2, H, hs2_tab[:], f"n{H}") for H in HALVES}
                S2 = {H: build_S(sl2, H, F32, f"u{H}") for H in HALVES}
                zmu_sb = zb.tile([P, NB, HID], F32, tag="zmu")
                zlv_sb = zb.tile([P, NB, HID], F32, tag="zlv")
                for b in range(NB):
                    ps = pagg2.tile([P, HID], F32, space="PSUM", tag="agg2")
                    ent = entries[b]
                    for i, (H, t) in enumerate(ent):
                        nc.tensor.matmul(out=ps[:], lhsT=S2[H](t), rhs=msg[H](t),
                                         start=(i == 0), stop=(i == len(ent) - 1))
                    a2 = l2sb.tile([P, HID], F32, tag="a2")
                    nc.vector.tensor_copy(out=a2[:], in_=ps[:])
                    a2T_ps = ptr.tile([HID, P], F32, space="PSUM", tag="a2T")
                    nc.tensor.transpose(out=a2T_ps[:], in_=a2[:], identity=ident[:])
                    a2T = l2sb.tile([HID, P], F32, tag="a2Ts")
                    nc.vector.tensor_copy(out=a2T[:], in_=a2T_ps[:])
                    for w_sb, z_sb, tg in ((wmu_sb, zmu_sb, "m"), (wlv_sb, zlv_sb, "l")):
                        zT_ps = pproj.tile([HID, P], F32, space="PSUM", tag="zT" + tg)
                        nc.tensor.matmul(out=zT_ps[:], lhsT=w_sb[:], rhs=a2T[:],
                                         start=True, stop=True)
                        zT = l2sb.tile([HID, P], F32, tag="zTs" + tg)
                        nc.vector.tensor_copy(out=zT[:], in_=zT_ps[:])
                        z_ps = pz.tile([P, HID], F32, space="PSUM", tag="z" + tg)
                        nc.tensor.transpose(out=z_ps[:], in_=zT[:],
                                            identity=ident[:HID, :HID])
                        nc.vector.tensor_scalar(
                            out=z_sb[:, b, :], in0=z_ps[:],
                            scalar1=dis_own[:, b:b + 1], scalar2=None,
                            op0=mybir.AluOpType.mult)
                        if has_bias:
                            off = HID if tg == "m" else 2 * HID
                            nc.vector.tensor_tensor(
                                out=z_sb[:, b, :], in0=z_sb[:, b, :],
                                in1=bias_bc[:, off:off + HID],
                                op=mybir.AluOpType.add)
                nc.sync.dma_start(out=zmu_out.ap()[:], in_=zmu_sb[:])
                nc.sync.dma_start(out=zlv_out.ap()[:], in_=zlv_sb[:])

    nc.compile()
    return nc


_CACHE = {}


def _get_program(edge_index, cfg, has_bias):
    layout = build_layout(edge_index, cfg)
    key = (layout["ntA"], layout["ntB"], has_bias)
    if key not in _CACHE:
        _CACHE[key] = (build_program(layout, cfg, has_bias), layout)
    else:
        _CACHE[key] = (_CACHE[key][0], layout)
    return _CACHE[key]


def make_in_maps(x, edge_index, W1, b1, Wmu, bmu, Wlv, blv, layout,
                 cfg=DEFAULT, has_bias=False):
    x = np.asarray(x, np.float32)
    xpad = np.zeros((cfg.NPAD, cfg.IN), np.float32)
    xpad[:x.shape[0]] = x
    xT = np.ascontiguousarray(xpad.T)
    base = dict(xT=xT, w1=np.asarray(W1, np.float32),
                wmu=np.asarray(Wmu, np.float32), wlv=np.asarray(Wlv, np.float32))
    if has_bias:
        base.update(b1=np.asarray(b1, np.float32).reshape(1, -1),
                    bmu=np.asarray(bmu, np.float32).reshape(1, -1),
                    blv=np.asarray(blv, np.float32).reshape(1, -1))
    maps = []
    for c in range(cfg.NCORES):
        m = dict(base)
        for k, v in layout["cores"][c].items():
            if v.size:
                m[k] = v
        maps.append(m)
    return maps


def unshard(results, cfg=DEFAULT):
    outs = []
    for name in ("zmu", "zlv"):
        blocks = [np.transpose(results[c][name], (1, 0, 2)).reshape(cfg.SHARD, cfg.HID)
                  for c in range(cfg.NCORES)]
        outs.append(np.concatenate(blocks, axis=0)[:cfg.N])
    return tuple(outs)


def kernel(x, edge_index, W1, b1, Wmu, bmu, Wlv, blv):
    cfg = DEFAULT
    has_bias = any(np.any(np.asarray(b)) for b in (b1, bmu, blv))
    nc, layout = _get_program(np.asarray(edge_index), cfg, has_bias)
    in_maps = make_in_maps(x, edge_index, W1, b1, Wmu, bmu, Wlv, blv,
                           layout, cfg, has_bias)
    res = run_bass_kernel_spmd(nc, in_maps, core_ids=list(range(cfg.NCORES)))
    return unshard(res.results, cfg)
